# revision 6
# baseline (speedup 1.0000x reference)
"""AttentionBlock Trainium2 Bass kernel (v3: ACT-bound design).

Full inputs -> shard batch over 8 NeuronCores (4 samples each) -> full output.

Design notes (per core, 4 samples):
  The softmax exp is irreducible work for the Activation engine:
  4 samples x 4 heads x 1024x1024 scores / 128 lanes = 131072 ACT columns
  ~= 112-135 us.  Everything else is scheduled to hide underneath it:

  - S = K^T Q per head runs as fp8(e4m3) DoubleRow matmuls (0.5 cyc/col):
    q8/k8 are stored [128, 2, 1024] with the second k-tile zeroed, so the
    natural channel layout is kept and the matmul still gets the 2x rate.
  - exp runs on ACT straight out of PSUM with scale=1/32 (the attention
    scale^2 folded into the activation), output written directly as fp8
    into the kt-slice layout the SV matmul wants.
  - SV (hout + Z via a fp8 ones-column in vt) uses real DoubleRow k-tiles
    (pairs of s-chunks), accumulating 4 heads into two PSUM accumulators
    (two heads share banks at partition offsets 0/64).
  - Z normalization: DVE reciprocal of the Z rows, tiny DMA gather to a
    [2, 1024] tile, PE broadcast, DVE multiply; proj consumes the two
    head-pair tiles directly via two accumulated matmuls with permuted
    weights (no hun reassembly DMA).
  - PSUM: 2-slot S' ring (4 banks) + 2 head-pair accumulators (4 banks);
    the broadcast/proj tiles reuse the accumulator tags.
  - The per-sample prologue (groupnorm, QKV, quantize) is emitted in
    pieces between attention units so every engine queue stays busy.
"""

import numpy as np
from contextlib import ExitStack

B, C, HW = 32, 128, 1024
NH, DH = 4, 32
GROUPS = 32
EPS = 1e-5
NCORES = 8
BPC = B // NCORES  # samples per core
NJP = 4  # s-chunk pairs

_CACHE = {}
TRACE = False
LAST_RESULT = None


def _patch_tile_waits(tile, mybir):
    """This walrus build encodes only one sync-wait slot per instruction;
    Tile can attach several. Split extra waits onto NoOps committed
    immediately before the instruction on the same engine queue
    (in-order => identical semantics)."""
    if getattr(tile.TileContext, "_mm_wait_patched", False):
        return
    orig = tile.TileContext._commit_instruction

    def patched(self, inst, lazy_reg_writes=True):
        si = getattr(inst, "sync_info", None)
        if (not isinstance(inst, mybir.InstNoOp) and si is not None
                and si.on_wait and len(si.on_wait) > 1):
            waits = list(si.on_wait)
            for w in waits[:-1]:
                nop = mybir.InstNoOp(
                    name=self.nc.get_next_instruction_name(),
                    engine=inst.engine,
                    bass_nofuse=True,
                    sync_info=mybir.SyncInfo(on_wait=[w], on_update=[]),
                )
                orig(self, nop, lazy_reg_writes=False)
            inst.sync_info = mybir.SyncInfo(
                on_wait=[waits[-1]], on_update=list(si.on_update))
        return orig(self, inst, lazy_reg_writes)

    tile.TileContext._commit_instruction = patched

    def patched_drain(self, tick_clock, wait_clock):
        # Collect end-of-kernel waits, then hand them out one per SP nop
        # (the drain keeps none); nops precede the teardown barrier on the
        # same queue, so semantics are preserved.
        self.nc.sync.drain()
        sink = self.nc.sync.nop(nofuse=True)
        wait_clock.add_sem_waits(
            sink.ins, tile.ScopedClock({None: tick_clock.global_clock}))
        si = sink.ins.sync_info
        waits = list(si.on_wait) if si and si.on_wait else []
        if len(waits) > 1:
            sink.ins.sync_info = mybir.SyncInfo(
                on_wait=[waits[0]], on_update=list(si.on_update))
            for w in waits[1:]:
                extra = self.nc.sync.nop(nofuse=True)
                extra.ins.sync_info = mybir.SyncInfo(on_wait=[w], on_update=[])

        self.nc.all_engine_barrier()
        assert self.sems is not None
        popped = self.nc._tile_sem_poison_stack.pop()
        assert popped is self._sem_poison
        self.nc.clear_and_free_semaphores(list(self.sems.allocated().values()))
        self.nc.all_engine_barrier()

    tile.TileContext._drain_and_barrier = patched_drain
    tile.TileContext._mm_wait_patched = True


def _build_nc():
    import concourse.bass as bass
    import concourse.tile as tile
    from concourse import mybir

    _patch_tile_waits(tile, mybir)

    f32 = mybir.dt.float32
    bf16 = mybir.dt.bfloat16
    nc = bass.Bass()

    x_d = nc.dram_tensor("x", [BPC, C, HW], f32, kind="ExternalInput")
    wq_d = nc.dram_tensor("wqT", [C, C], bf16, kind="ExternalInput")
    wk_d = nc.dram_tensor("wkT", [C, C], bf16, kind="ExternalInput")
    wv_d = nc.dram_tensor("wvT", [C, C], bf16, kind="ExternalInput")
    pjA_d = nc.dram_tensor("pjA", [C, C], bf16, kind="ExternalInput")
    pjB_d = nc.dram_tensor("pjB", [C, C], bf16, kind="ExternalInput")
    ebc_d = nc.dram_tensor("ebcP", [2, C], bf16, kind="ExternalInput")
    pjb_d = nc.dram_tensor("pjb", [C, 1], f32, kind="ExternalInput")
    nw_d = nc.dram_tensor("nw", [C, 1], f32, kind="ExternalInput")
    nb_d = nc.dram_tensor("nb", [C, 1], f32, kind="ExternalInput")
    g1_d = nc.dram_tensor("g1", [C, GROUPS], f32, kind="ExternalInput")
    g2_d = nc.dram_tensor("g2", [GROUPS, C], f32, kind="ExternalInput")
    y_d = nc.dram_tensor("y", [BPC, C, HW], f32, kind="ExternalOutput")

    with tile.TileContext(nc) as tc:
        with ExitStack() as ctx:
            _body(ctx, tc, mybir, bass,
                  x_d, wq_d, wk_d, wv_d, pjA_d, pjB_d, ebc_d, pjb_d,
                  nw_d, nb_d, g1_d, g2_d, y_d)
    return nc


def _body(ctx, tc, mybir, bass,
          x_d, wq_d, wk_d, wv_d, pjA_d, pjB_d, ebc_d, pjb_d,
          nw_d, nb_d, g1_d, g2_d, y_d):
    nc = tc.nc
    f32 = mybir.dt.float32
    bf16 = mybir.dt.bfloat16
    fp8 = mybir.dt.float8e4
    AF = mybir.ActivationFunctionType
    OP = mybir.AluOpType
    DR = mybir.MatmulPerfMode.DoubleRow
    NHALF = HW // 2

    const = ctx.enter_context(tc.tile_pool(name="const", bufs=1))
    pers = ctx.enter_context(tc.tile_pool(name="pers", bufs=1))
    sb_x = ctx.enter_context(tc.tile_pool(name="sb_x", bufs=2))
    sb_a = ctx.enter_context(tc.tile_pool(name="sb_a", bufs=4))
    sb_h = ctx.enter_context(tc.tile_pool(name="sb_h", bufs=2))
    sb_sm = ctx.enter_context(tc.tile_pool(name="sb_sm", bufs=2))
    # PSUM: ring 2 x [128,1024]f32 (4 banks) + ha01/ha23 (4 banks) = 8
    ps_ring = ctx.enter_context(tc.tile_pool(name="ps_ring", bufs=2,
                                             space="PSUM"))
    ps_acc = ctx.enter_context(tc.tile_pool(name="ps_acc", bufs=1,
                                            space="PSUM"))

    # ---- constants ----
    wq_sb = const.tile([C, C], bf16, tag="wq")
    wk_sb = const.tile([C, C], bf16, tag="wk")
    wv_sb = const.tile([C, C], bf16, tag="wv")
    pjA_sb = const.tile([C, C], bf16, tag="pjA")
    pjB_sb = const.tile([C, C], bf16, tag="pjB")
    ebc_sb = const.tile([2, C], bf16, tag="ebc")
    pjb_sb = const.tile([C, 1], f32, tag="pjb")
    nw_sb = const.tile([C, 1], f32, tag="nw")
    nb_sb = const.tile([C, 1], f32, tag="nb")
    g1_sb = const.tile([C, GROUPS], f32, tag="g1")
    g2_sb = const.tile([GROUPS, C], f32, tag="g2")
    for dst, src in ((wq_sb, wq_d), (wk_sb, wk_d), (wv_sb, wv_d),
                     (pjA_sb, pjA_d), (pjB_sb, pjB_d), (ebc_sb, ebc_d),
                     (pjb_sb, pjb_d), (nw_sb, nw_d), (nb_sb, nb_d),
                     (g1_sb, g1_d), (g2_sb, g2_d)):
        nc.sync.dma_start(out=dst, in_=src[:])
    eps_sb = const.tile([GROUPS, 1], f32, tag="eps")
    nc.vector.memset(eps_sb, EPS)

    # persistent fp8 operand tiles (double-buffered by sample parity);
    # the kt=1 half of q8/k8 and the ones column of vt8 are write-once
    q8s = [pers.tile([C, 2, HW], fp8, tag="q8_%d" % i, name="q8_%d" % i)
           for i in range(2)]
    k8s = [pers.tile([C, 2, HW], fp8, tag="k8_%d" % i, name="k8_%d" % i)
           for i in range(2)]
    # vtP[p, pair, j, kt, col]: kt is the head-in-pair index; plane kt=0
    # holds head 2*pair in cols 0-63 (v cols 0-31, Z ones at 32), plane
    # kt=1 holds head 2*pair+1 in cols 64-127 (ones at 96); everything
    # else is zero, so one DoubleRow matmul computes both heads' SV with
    # full 128-partition output and no column tiling.
    vtPs = [pers.tile([C, 2, 8, 2, C], fp8, tag="vtP_%d" % i,
                      name="vtP_%d" % i) for i in range(2)]
    for i in range(2):
        nc.vector.memset(q8s[i][:, 1, :], 0.0)
        nc.vector.memset(k8s[i][:, 1, :], 0.0)
        nc.vector.memset(vtPs[i], 0.0)
        nc.vector.memset(vtPs[i][:, :, :, 0, 32:33], 1.0)
        nc.vector.memset(vtPs[i][:, :, :, 1, 96:97], 1.0)

    xb2s = {}

    # ---------------- prologue pieces ----------------
    def prologue_a(b):
        # load + groupnorm stats + affine -> xn
        x_sb = sb_x.tile([C, HW], f32, tag="x")
        nc.sync.dma_start(out=x_sb, in_=x_d[b])
        xb2 = sb_x.tile([C, HW], f32, tag="xb2")
        nc.vector.tensor_scalar(out=xb2, in0=x_sb, scalar1=pjb_sb,
                                scalar2=None, op0=OP.add)
        xb2s[b] = xb2

        st6 = sb_sm.tile([C, 2, 6], f32, tag="st6")
        nc.vector.bn_stats(out=st6[:, 0, :], in_=x_sb[:, 0:512])
        nc.vector.bn_stats(out=st6[:, 1, :], in_=x_sb[:, 512:1024])
        mv = sb_sm.tile([C, 2], f32, tag="mv")
        nc.vector.bn_aggr(out=mv, in_=st6)
        # s2 = [mean_c, mean_c^2 + var_c]
        s2 = sb_sm.tile([C, 2], f32, tag="s2")
        nc.vector.tensor_copy(out=s2[:, 0:1], in_=mv[:, 0:1])
        nc.vector.tensor_mul(out=s2[:, 1:2], in0=mv[:, 0:1], in1=mv[:, 0:1])
        nc.vector.tensor_add(out=s2[:, 1:2], in0=s2[:, 1:2], in1=mv[:, 1:2])
        gp = ps_ring.tile([C, HW], f32, tag="s")
        nc.tensor.matmul(gp[0:GROUPS, 0:2], g1_sb, s2, start=True, stop=True)
        gs = sb_sm.tile([GROUPS, 2], f32, tag="gs")
        nc.vector.tensor_copy(out=gs, in_=gp[0:GROUPS, 0:2])
        # vv = [mu_g, rstd_g]; rstd = exp(-0.5*ln(var+eps))
        vv = sb_sm.tile([GROUPS, 2], f32, tag="vv")
        nc.vector.tensor_mul(out=vv[:, 0:1], in0=gs[:, 0:1], in1=gs[:, 0:1])
        nc.vector.tensor_tensor(out=vv[:, 1:2], in0=gs[:, 1:2], in1=vv[:, 0:1],
                                op=OP.subtract)
        nc.scalar.activation(out=vv[:, 1:2], in_=vv[:, 1:2], func=AF.Ln,
                             bias=eps_sb, scale=1.0)
        nc.scalar.activation(out=vv[:, 1:2], in_=vv[:, 1:2], func=AF.Exp,
                             bias=0.0, scale=-0.5)
        nc.vector.tensor_copy(out=vv[:, 0:1], in_=gs[:, 0:1])
        bc = ps_ring.tile([C, HW], f32, tag="s")
        nc.tensor.matmul(bc[0:C, 0:2], g2_sb, vv, start=True, stop=True)
        aff = sb_sm.tile([C, 2], f32, tag="aff")
        nc.vector.tensor_mul(out=aff[:, 0:1], in0=nw_sb, in1=bc[:, 1:2])
        nc.vector.tensor_mul(out=aff[:, 1:2], in0=bc[:, 0:1], in1=aff[:, 0:1])
        nc.vector.tensor_tensor(out=aff[:, 1:2], in0=nb_sb, in1=aff[:, 1:2],
                                op=OP.subtract)
        xn = sb_x.tile([C, HW], bf16, tag="xn")
        nc.vector.tensor_scalar(out=xn, in0=x_sb,
                                scalar1=aff[:, 0:1], scalar2=aff[:, 1:2],
                                op0=OP.mult, op1=OP.add)
        return xn

    def prologue_q(b, xn):
        qp = ps_ring.tile([C, HW], f32, tag="s")
        for n in range(2):
            sl = slice(n * NHALF, (n + 1) * NHALF)
            nc.tensor.matmul(qp[:, sl], wq_sb, xn[:, sl], start=True, stop=True)
        nc.vector.tensor_copy(out=q8s[b % 2][:, 0, :], in_=qp)

    def prologue_k(b, xn):
        kp = ps_ring.tile([C, HW], f32, tag="s")
        for n in range(2):
            sl = slice(n * NHALF, (n + 1) * NHALF)
            nc.tensor.matmul(kp[:, sl], wk_sb, xn[:, sl], start=True, stop=True)
        nc.vector.tensor_copy(out=k8s[b % 2][:, 0, :], in_=kp)

    def prologue_v(b, xn):
        vp = ps_ring.tile([C, HW], f32, tag="s")
        vtP = vtPs[b % 2]
        for j in range(8):
            nc.tensor.matmul(vp[:, j * 128:(j + 1) * 128],
                             xn[:, j * 128:(j + 1) * 128], wv_sb,
                             start=True, stop=True)
        for j in range(8):
            src_v = vp[:, j * 128:(j + 1) * 128].rearrange(
                "p (pr hi d) -> p pr hi d", hi=2, d=DH)
            nc.vector.tensor_copy(out=vtP[:, :, j, 0, 0:DH],
                                  in_=src_v[:, :, 0, :])
            nc.vector.tensor_copy(out=vtP[:, :, j, 1, 64:64 + DH],
                                  in_=src_v[:, :, 1, :])

    # ---------------- attention unit ----------------
    ha_cur = {}

    def unit(b, pair, j):
        q8, k8, vtP = q8s[b % 2], k8s[b % 2], vtPs[b % 2]
        aP = sb_a.tile([C, 2, HW], fp8, tag="a", name="aP")
        for hi in range(2):
            h = 2 * pair + hi
            sp = ps_ring.tile([C, HW], f32, tag="s", name="sp")
            for n in range(2):
                sl = slice(n * NHALF, (n + 1) * NHALF)
                nc.tensor.matmul(
                    sp[:, sl],
                    k8[32 * h:32 * h + 32, :, 128 * j:128 * (j + 1)],
                    q8[32 * h:32 * h + 32, :, sl],
                    start=True, stop=True,
                    tile_position=(32 * h, 0),
                    perf_mode=DR)
            nc.scalar.activation(out=aP[:, hi, :], in_=sp, func=AF.Exp,
                                 bias=0.0, scale=1.0 / 32.0)
        if j == 0:
            ha_cur[pair] = ps_acc.tile([C, HW], f32,
                                       tag="ha01" if pair == 0 else "ha23",
                                       name="ha%d" % pair)
        ha = ha_cur[pair]
        for n in range(2):
            sl = slice(n * NHALF, (n + 1) * NHALF)
            nc.tensor.matmul(
                ha[:, sl],
                vtP[:, pair, j, :, :],
                aP[:, :, sl],
                start=(j == 0), stop=(j == 7),
                tile_position=(0, 0),
                perf_mode=DR)

    # ---------------- tails ----------------
    hvns = {}

    def pair_tail(b, pair):
        ha = ha_cur[pair]
        hv = sb_h.tile([C, HW], bf16, tag="hv%d" % pair)
        nc.vector.tensor_copy(out=hv, in_=ha)
        zr = sb_h.tile([C, HW], bf16, tag="zr%d" % pair)
        with nc.allow_low_precision(reason="1/Z in bf16; gate 2e-2"):
            nc.vector.reciprocal(out=zr[32:33, :], in_=ha[32:33, :])
            nc.vector.reciprocal(out=zr[96:97, :], in_=ha[96:97, :])
        ral = sb_h.tile([2, HW], bf16, tag="ral%d" % pair)
        nc.sync.dma_start(out=ral[0:1, :], in_=zr[32:33, :])
        nc.sync.dma_start(out=ral[1:2, :], in_=zr[96:97, :])
        # broadcast 1/Z to the hv row layout, reusing the accumulator banks
        rb = ps_acc.tile([C, HW], f32, tag="ha01" if pair == 0 else "ha23")
        for n in range(2):
            sl = slice(n * NHALF, (n + 1) * NHALF)
            nc.tensor.matmul(rb[:, sl], ebc_sb, ral[:, sl],
                             start=True, stop=True)
        hvn = sb_h.tile([C, HW], bf16, tag="hvn%d" % pair)
        nc.vector.tensor_mul(out=hvn, in0=hv, in1=rb)
        hvns[(b, pair)] = hvn

    def end_tail(b):
        pp = ps_acc.tile([C, HW], f32, tag="ha23")
        for pair in range(2):
            pj = pjA_sb if pair == 0 else pjB_sb
            hvn = hvns.pop((b, pair))
            for n in range(2):
                sl = slice(n * NHALF, (n + 1) * NHALF)
                nc.tensor.matmul(pp[:, sl], pj, hvn[:, sl],
                                 start=(pair == 0), stop=(pair == 1))
        out_sb = sb_x.tile([C, HW], f32, tag="out")
        nc.vector.tensor_add(out=out_sb, in0=pp, in1=xb2s.pop(b))
        nc.sync.dma_start(out=y_d[b], in_=out_sb)

    # ---------------- schedule ----------------
    xns = {0: prologue_a(0)}
    prologue_q(0, xns[0])
    prologue_k(0, xns[0])
    prologue_v(0, xns[0])

    for b in range(BPC):
        for pair in range(2):
            for j in range(8):
                unit(b, pair, j)
                key = (pair, j)
                if key == (0, 0) and b > 0:
                    pair_tail(b - 1, 1)
                elif key == (0, 1) and b > 0:
                    end_tail(b - 1)
                elif key == (0, 5) and b + 1 < BPC:
                    xns[b + 1] = prologue_a(b + 1)
                elif key == (0, 7):
                    pair_tail(b, 0)
                elif key == (1, 1) and b + 1 < BPC:
                    prologue_q(b + 1, xns[b + 1])
                elif key == (1, 3) and b + 1 < BPC:
                    prologue_k(b + 1, xns[b + 1])
                elif key == (1, 5) and b + 1 < BPC:
                    prologue_v(b + 1, xns.pop(b + 1))
    pair_tail(BPC - 1, 1)
    end_tail(BPC - 1)


def _get_nc():
    if "nc" not in _CACHE:
        _CACHE["nc"] = _build_nc()
    return _CACHE["nc"]


def _host_prep(inputs):
    import ml_dtypes
    bf = ml_dtypes.bfloat16

    x = np.ascontiguousarray(
        np.asarray(inputs["x"], np.float32).reshape(B, C, HW))
    qkv_w = np.asarray(inputs["qkv_w"], np.float32)
    proj_w = np.asarray(inputs["proj_w"], np.float32)
    proj_b = np.asarray(inputs["proj_b"], np.float32)
    norm_w = np.asarray(inputs["norm_w"], np.float32)
    norm_b = np.asarray(inputs["norm_b"], np.float32)

    w3 = qkv_w.reshape(NH, 3, DH, C)  # rows: h*96 + which*32 + d
    wq = w3[:, 0].reshape(C, C)
    wk = w3[:, 1].reshape(C, C)
    wv = w3[:, 2].reshape(C, C)
    wqT = np.ascontiguousarray(wq.T).astype(bf)  # scale^2 applied in exp
    wkT = np.ascontiguousarray(wk.T).astype(bf)
    wvT = np.ascontiguousarray(wv.T).astype(bf)

    # proj weights permuted to consume the head-pair PSUM row layout:
    # rows 0-31 = first head of pair, rows 64-95 = second head.
    pjA = np.zeros((C, C), np.float32)
    pjB = np.zeros((C, C), np.float32)
    pjA[0:32, :] = proj_w[:, 0:32].T
    pjA[64:96, :] = proj_w[:, 32:64].T
    pjB[0:32, :] = proj_w[:, 64:96].T
    pjB[64:96, :] = proj_w[:, 96:128].T
    pjA = pjA.astype(bf)
    pjB = pjB.astype(bf)

    ebcP = np.zeros((2, C), np.float32)
    ebcP[0, 0:32] = 1.0
    ebcP[1, 64:96] = 1.0
    ebcP = ebcP.astype(bf)

    g1 = np.zeros((C, GROUPS), np.float32)
    g1[np.arange(C), np.arange(C) // 4] = 0.25
    g2 = np.zeros((GROUPS, C), np.float32)
    g2[np.arange(C) // 4, np.arange(C)] = 1.0

    params = dict(
        wqT=wqT, wkT=wkT, wvT=wvT, pjA=pjA, pjB=pjB, ebcP=ebcP,
        pjb=np.ascontiguousarray(proj_b[:, None]),
        nw=np.ascontiguousarray(norm_w[:, None]),
        nb=np.ascontiguousarray(norm_b[:, None]),
        g1=g1, g2=g2,
    )
    in_maps = []
    for i in range(NCORES):
        m = dict(params)
        m["x"] = np.ascontiguousarray(x[i * BPC:(i + 1) * BPC])
        in_maps.append(m)
    return in_maps


def kernel(**inputs):
    global LAST_RESULT
    from concourse.bass_utils import run_bass_kernel_spmd
    in_maps = _host_prep(inputs)
    nc = _get_nc()
    res = run_bass_kernel_spmd(nc, in_maps, list(range(NCORES)), trace=TRACE)
    LAST_RESULT = res
    y = np.concatenate([res.results[i]["y"] for i in range(NCORES)], axis=0)
    return y.reshape(B, C, 32, 32)


# revision 7
# speedup vs baseline: 1.2216x; 1.2216x over previous
"""AttentionBlock Trainium2 Bass kernel (v3: ACT-bound design).

Full inputs -> shard batch over 8 NeuronCores (4 samples each) -> full output.

Design notes (per core, 4 samples):
  The softmax exp is irreducible work for the Activation engine:
  4 samples x 4 heads x 1024x1024 scores / 128 lanes = 131072 ACT columns
  ~= 112-135 us.  Everything else is scheduled to hide underneath it:

  - S = K^T Q per head runs as fp8(e4m3) DoubleRow matmuls (0.5 cyc/col):
    q8/k8 are stored [128, 2, 1024] with the second k-tile zeroed, so the
    natural channel layout is kept and the matmul still gets the 2x rate.
  - exp runs on ACT straight out of PSUM with scale=1/32 (the attention
    scale^2 folded into the activation), output written directly as fp8
    into the kt-slice layout the SV matmul wants.
  - SV (hout + Z via a fp8 ones-column in vt) uses real DoubleRow k-tiles
    (pairs of s-chunks), accumulating 4 heads into two PSUM accumulators
    (two heads share banks at partition offsets 0/64).
  - Z normalization: DVE reciprocal of the Z rows, tiny DMA gather to a
    [2, 1024] tile, PE broadcast, DVE multiply; proj consumes the two
    head-pair tiles directly via two accumulated matmuls with permuted
    weights (no hun reassembly DMA).
  - PSUM: 2-slot S' ring (4 banks) + 2 head-pair accumulators (4 banks);
    the broadcast/proj tiles reuse the accumulator tags.
  - The per-sample prologue (groupnorm, QKV, quantize) is emitted in
    pieces between attention units so every engine queue stays busy.
"""

import numpy as np
from contextlib import ExitStack

B, C, HW = 32, 128, 1024
NH, DH = 4, 32
GROUPS = 32
EPS = 1e-5
NCORES = 8
BPC = B // NCORES  # samples per core
NJP = 4  # s-chunk pairs

_CACHE = {}
TRACE = False
LAST_RESULT = None


def _patch_tile_waits(tile, mybir):
    """This walrus build encodes only one sync-wait slot per instruction;
    Tile can attach several. Split extra waits onto NoOps committed
    immediately before the instruction on the same engine queue
    (in-order => identical semantics)."""
    if getattr(tile.TileContext, "_mm_wait_patched", False):
        return
    orig = tile.TileContext._commit_instruction

    def patched(self, inst, lazy_reg_writes=True):
        si = getattr(inst, "sync_info", None)
        if (not isinstance(inst, mybir.InstNoOp) and si is not None
                and si.on_wait and len(si.on_wait) > 1):
            waits = list(si.on_wait)
            for w in waits[:-1]:
                nop = mybir.InstNoOp(
                    name=self.nc.get_next_instruction_name(),
                    engine=inst.engine,
                    bass_nofuse=True,
                    sync_info=mybir.SyncInfo(on_wait=[w], on_update=[]),
                )
                orig(self, nop, lazy_reg_writes=False)
            inst.sync_info = mybir.SyncInfo(
                on_wait=[waits[-1]], on_update=list(si.on_update))
        return orig(self, inst, lazy_reg_writes)

    tile.TileContext._commit_instruction = patched

    def patched_drain(self, tick_clock, wait_clock):
        # Collect end-of-kernel waits, then hand them out one per SP nop
        # (the drain keeps none); nops precede the teardown barrier on the
        # same queue, so semantics are preserved.
        self.nc.sync.drain()
        sink = self.nc.sync.nop(nofuse=True)
        wait_clock.add_sem_waits(
            sink.ins, tile.ScopedClock({None: tick_clock.global_clock}))
        si = sink.ins.sync_info
        waits = list(si.on_wait) if si and si.on_wait else []
        if len(waits) > 1:
            sink.ins.sync_info = mybir.SyncInfo(
                on_wait=[waits[0]], on_update=list(si.on_update))
            for w in waits[1:]:
                extra = self.nc.sync.nop(nofuse=True)
                extra.ins.sync_info = mybir.SyncInfo(on_wait=[w], on_update=[])

        self.nc.all_engine_barrier()
        assert self.sems is not None
        popped = self.nc._tile_sem_poison_stack.pop()
        assert popped is self._sem_poison
        self.nc.clear_and_free_semaphores(list(self.sems.allocated().values()))
        self.nc.all_engine_barrier()

    tile.TileContext._drain_and_barrier = patched_drain
    tile.TileContext._mm_wait_patched = True


def _build_nc():
    import concourse.bass as bass
    import concourse.tile as tile
    from concourse import mybir

    _patch_tile_waits(tile, mybir)

    f32 = mybir.dt.float32
    bf16 = mybir.dt.bfloat16
    nc = bass.Bass()

    x_d = nc.dram_tensor("x", [BPC, C, HW], f32, kind="ExternalInput")
    wq_d = nc.dram_tensor("wqT", [C, C], bf16, kind="ExternalInput")
    wk_d = nc.dram_tensor("wkT", [C, C], bf16, kind="ExternalInput")
    wv_d = nc.dram_tensor("wvT", [C, C], bf16, kind="ExternalInput")
    pjA_d = nc.dram_tensor("pjA", [C, C], bf16, kind="ExternalInput")
    pjB_d = nc.dram_tensor("pjB", [C, C], bf16, kind="ExternalInput")
    ebc_d = nc.dram_tensor("ebcP", [2, C], bf16, kind="ExternalInput")
    pjb_d = nc.dram_tensor("pjb", [C, 1], f32, kind="ExternalInput")
    nw_d = nc.dram_tensor("nw", [C, 1], f32, kind="ExternalInput")
    nb_d = nc.dram_tensor("nb", [C, 1], f32, kind="ExternalInput")
    g1_d = nc.dram_tensor("g1", [C, GROUPS], f32, kind="ExternalInput")
    g2_d = nc.dram_tensor("g2", [GROUPS, C], f32, kind="ExternalInput")
    y_d = nc.dram_tensor("y", [BPC, C, HW], f32, kind="ExternalOutput")

    with tile.TileContext(nc) as tc:
        with ExitStack() as ctx:
            _body(ctx, tc, mybir, bass,
                  x_d, wq_d, wk_d, wv_d, pjA_d, pjB_d, ebc_d, pjb_d,
                  nw_d, nb_d, g1_d, g2_d, y_d)
    return nc


def _body(ctx, tc, mybir, bass,
          x_d, wq_d, wk_d, wv_d, pjA_d, pjB_d, ebc_d, pjb_d,
          nw_d, nb_d, g1_d, g2_d, y_d):
    nc = tc.nc
    f32 = mybir.dt.float32
    bf16 = mybir.dt.bfloat16
    fp8 = mybir.dt.float8e4
    AF = mybir.ActivationFunctionType
    OP = mybir.AluOpType
    DR = mybir.MatmulPerfMode.DoubleRow
    NHALF = HW // 2

    const = ctx.enter_context(tc.tile_pool(name="const", bufs=1))
    pers = ctx.enter_context(tc.tile_pool(name="pers", bufs=1))
    sb_x = ctx.enter_context(tc.tile_pool(name="sb_x", bufs=2))
    sb_a = ctx.enter_context(tc.tile_pool(name="sb_a", bufs=4))
    sb_qk = ctx.enter_context(tc.tile_pool(name="sb_qk", bufs=2))
    sb_h = ctx.enter_context(tc.tile_pool(name="sb_h", bufs=2))
    sb_sm = ctx.enter_context(tc.tile_pool(name="sb_sm", bufs=2))
    # PSUM: ring 2 x [128,1024]f32 (4 banks) + ha01/ha23 (4 banks) = 8
    ps_ring = ctx.enter_context(tc.tile_pool(name="ps_ring", bufs=2,
                                             space="PSUM"))
    ps_acc = ctx.enter_context(tc.tile_pool(name="ps_acc", bufs=1,
                                            space="PSUM"))

    # ---- constants ----
    wq_sb = const.tile([C, C], bf16, tag="wq")
    wk_sb = const.tile([C, C], bf16, tag="wk")
    wv_sb = const.tile([C, C], bf16, tag="wv")
    pjA_sb = const.tile([C, C], bf16, tag="pjA")
    pjB_sb = const.tile([C, C], bf16, tag="pjB")
    ebc_sb = const.tile([2, C], bf16, tag="ebc")
    pjb_sb = const.tile([C, 1], f32, tag="pjb")
    nw_sb = const.tile([C, 1], f32, tag="nw")
    nb_sb = const.tile([C, 1], f32, tag="nb")
    g1_sb = const.tile([C, GROUPS], f32, tag="g1")
    g2_sb = const.tile([GROUPS, C], f32, tag="g2")
    for dst, src in ((wq_sb, wq_d), (wk_sb, wk_d), (wv_sb, wv_d),
                     (pjA_sb, pjA_d), (pjB_sb, pjB_d), (ebc_sb, ebc_d),
                     (pjb_sb, pjb_d), (nw_sb, nw_d), (nb_sb, nb_d),
                     (g1_sb, g1_d), (g2_sb, g2_d)):
        nc.sync.dma_start(out=dst, in_=src[:])
    eps_sb = const.tile([GROUPS, 1], f32, tag="eps")
    nc.vector.memset(eps_sb, EPS)

    # persistent fp8 operand tiles (double-buffered by sample parity);
    # the kt=1 half of q8/k8 and the ones column of vt8 are write-once
    # vtP[p, pair, j, kt, col]: kt is the head-in-pair index; plane kt=0
    # holds head 2*pair in cols 0-63 (v cols 0-31, Z ones at 32), plane
    # kt=1 holds head 2*pair+1 in cols 64-127 (ones at 96); everything
    # else is zero, so one DoubleRow matmul computes both heads' SV with
    # full 128-partition output and no column tiling.
    vtPs = [pers.tile([C, 2, 8, 2, C], fp8, tag="vtP_%d" % i,
                      name="vtP_%d" % i) for i in range(2)]
    for i in range(2):
        nc.vector.memset(vtPs[i], 0.0)
        nc.vector.memset(vtPs[i][:, :, :, 0, 32:33], 1.0)
        nc.vector.memset(vtPs[i][:, :, :, 1, 96:97], 1.0)

    xb2s = {}

    # ---------------- prologue pieces ----------------
    def prologue_a(b):
        # load + groupnorm stats + affine -> xn
        x_sb = sb_x.tile([C, HW], f32, tag="x")
        nc.sync.dma_start(out=x_sb, in_=x_d[b])
        xb2 = sb_x.tile([C, HW], f32, tag="xb2")
        nc.vector.tensor_scalar(out=xb2, in0=x_sb, scalar1=pjb_sb,
                                scalar2=None, op0=OP.add)
        xb2s[b] = xb2

        st6 = sb_sm.tile([C, 2, 6], f32, tag="st6")
        nc.vector.bn_stats(out=st6[:, 0, :], in_=x_sb[:, 0:512])
        nc.vector.bn_stats(out=st6[:, 1, :], in_=x_sb[:, 512:1024])
        mv = sb_sm.tile([C, 2], f32, tag="mv")
        nc.vector.bn_aggr(out=mv, in_=st6)
        # s2 = [mean_c, mean_c^2 + var_c]
        s2 = sb_sm.tile([C, 2], f32, tag="s2")
        nc.vector.tensor_copy(out=s2[:, 0:1], in_=mv[:, 0:1])
        nc.vector.tensor_mul(out=s2[:, 1:2], in0=mv[:, 0:1], in1=mv[:, 0:1])
        nc.vector.tensor_add(out=s2[:, 1:2], in0=s2[:, 1:2], in1=mv[:, 1:2])
        gp = ps_ring.tile([C, HW], f32, tag="s")
        nc.tensor.matmul(gp[0:GROUPS, 0:2], g1_sb, s2, start=True, stop=True)
        gs = sb_sm.tile([GROUPS, 2], f32, tag="gs")
        nc.vector.tensor_copy(out=gs, in_=gp[0:GROUPS, 0:2])
        # vv = [mu_g, rstd_g]; rstd = exp(-0.5*ln(var+eps))
        vv = sb_sm.tile([GROUPS, 2], f32, tag="vv")
        nc.vector.tensor_mul(out=vv[:, 0:1], in0=gs[:, 0:1], in1=gs[:, 0:1])
        nc.vector.tensor_tensor(out=vv[:, 1:2], in0=gs[:, 1:2], in1=vv[:, 0:1],
                                op=OP.subtract)
        nc.scalar.activation(out=vv[:, 1:2], in_=vv[:, 1:2], func=AF.Ln,
                             bias=eps_sb, scale=1.0)
        nc.scalar.activation(out=vv[:, 1:2], in_=vv[:, 1:2], func=AF.Exp,
                             bias=0.0, scale=-0.5)
        nc.vector.tensor_copy(out=vv[:, 0:1], in_=gs[:, 0:1])
        bc = ps_ring.tile([C, HW], f32, tag="s")
        nc.tensor.matmul(bc[0:C, 0:2], g2_sb, vv, start=True, stop=True)
        aff = sb_sm.tile([C, 2], f32, tag="aff")
        nc.vector.tensor_mul(out=aff[:, 0:1], in0=nw_sb, in1=bc[:, 1:2])
        nc.vector.tensor_mul(out=aff[:, 1:2], in0=bc[:, 0:1], in1=aff[:, 0:1])
        nc.vector.tensor_tensor(out=aff[:, 1:2], in0=nb_sb, in1=aff[:, 1:2],
                                op=OP.subtract)
        xn = sb_x.tile([C, HW], bf16, tag="xn")
        nc.vector.tensor_scalar(out=xn, in0=x_sb,
                                scalar1=aff[:, 0:1], scalar2=aff[:, 1:2],
                                op0=OP.mult, op1=OP.add)
        return xn

    qks = {}

    def prologue_q(b, xn):
        qp = ps_ring.tile([C, HW], f32, tag="s")
        for n in range(2):
            sl = slice(n * NHALF, (n + 1) * NHALF)
            nc.tensor.matmul(qp[:, sl], wq_sb, xn[:, sl], start=True, stop=True)
        q_sb = sb_qk.tile([C, HW], bf16, tag="q")
        nc.vector.tensor_copy(out=q_sb, in_=qp)
        qks[(b, "q")] = q_sb

    def prologue_k(b, xn):
        kp = ps_ring.tile([C, HW], f32, tag="s")
        for n in range(2):
            sl = slice(n * NHALF, (n + 1) * NHALF)
            nc.tensor.matmul(kp[:, sl], wk_sb, xn[:, sl], start=True, stop=True)
        k_sb = sb_qk.tile([C, HW], bf16, tag="k")
        nc.vector.tensor_copy(out=k_sb, in_=kp)
        qks[(b, "k")] = k_sb

    def prologue_v(b, xn):
        vp = ps_ring.tile([C, HW], f32, tag="s")
        vtP = vtPs[b % 2]
        for j in range(8):
            nc.tensor.matmul(vp[:, j * 128:(j + 1) * 128],
                             xn[:, j * 128:(j + 1) * 128], wv_sb,
                             start=True, stop=True)
        for j in range(8):
            src_v = vp[:, j * 128:(j + 1) * 128].rearrange(
                "p (pr hi d) -> p pr hi d", hi=2, d=DH)
            nc.vector.tensor_copy(out=vtP[:, :, j, 0, 0:DH],
                                  in_=src_v[:, :, 0, :])
            nc.vector.tensor_copy(out=vtP[:, :, j, 1, 64:64 + DH],
                                  in_=src_v[:, :, 1, :])

    # ---------------- attention unit ----------------
    ha_cur = {}

    def unit(b, pair, j):
        q_sb, k_sb = qks[(b, "q")], qks[(b, "k")]
        vtP = vtPs[b % 2]
        aP = sb_a.tile([C, 2, HW], fp8, tag="a", name="aP")
        sps = [ps_ring.tile([C, HW], f32, tag="s", name="sp%d" % hi)
               for hi in range(2)]
        # issue the two heads' S' matmuls interleaved: they sit in
        # different 32-row PE bands (tile_position) and stream
        # concurrently in the array
        for n in range(2):
            sl = slice(n * NHALF, (n + 1) * NHALF)
            for hi in range(2):
                h = 2 * pair + hi
                hp = slice(32 * h, 32 * h + 32)
                nc.tensor.matmul(
                    sps[hi][:, sl],
                    k_sb[hp, 128 * j:128 * (j + 1)],
                    q_sb[hp, sl],
                    start=True, stop=True,
                    tile_position=(32 * h, 0))
        for hi in range(2):
            nc.scalar.activation(out=aP[:, hi, :], in_=sps[hi], func=AF.Exp,
                                 bias=0.0, scale=1.0 / 32.0)
        if j == 0:
            ha_cur[pair] = ps_acc.tile([C, HW], f32,
                                       tag="ha01" if pair == 0 else "ha23",
                                       name="ha%d" % pair)
        ha = ha_cur[pair]
        for n in range(2):
            sl = slice(n * NHALF, (n + 1) * NHALF)
            nc.tensor.matmul(
                ha[:, sl],
                vtP[:, pair, j, :, :],
                aP[:, :, sl],
                start=(j == 0), stop=(j == 7),
                tile_position=(0, 0),
                perf_mode=DR)

    # ---------------- tails ----------------
    hvns = {}

    def pair_tail(b, pair):
        ha = ha_cur[pair]
        hv = sb_h.tile([C, HW], bf16, tag="hv%d" % pair)
        nc.vector.tensor_copy(out=hv, in_=ha)
        # 1/Z: spread the two Z rows across partitions so the DVE
        # reciprocal runs at free-size 16 instead of 1024
        zp = sb_h.tile([C, 2, 8], bf16, tag="zp%d" % pair)
        nc.sync.dma_start(out=zp[:, 0, :], in_=hv[32:33, :])
        nc.sync.dma_start(out=zp[:, 1, :], in_=hv[96:97, :])
        rp = sb_h.tile([C, 2, 8], bf16, tag="rp%d" % pair)
        with nc.allow_low_precision(reason="1/Z in bf16; gate 2e-2"):
            nc.vector.reciprocal(out=rp, in_=zp)
        ral = sb_h.tile([2, HW], bf16, tag="ral%d" % pair)
        nc.sync.dma_start(out=ral[0:1, :], in_=rp[:, 0, :])
        nc.sync.dma_start(out=ral[1:2, :], in_=rp[:, 1, :])
        # broadcast 1/Z to the hv row layout, reusing the accumulator banks
        rb = ps_acc.tile([C, HW], f32, tag="ha01" if pair == 0 else "ha23")
        for n in range(2):
            sl = slice(n * NHALF, (n + 1) * NHALF)
            nc.tensor.matmul(rb[:, sl], ebc_sb, ral[:, sl],
                             start=True, stop=True)
        hvn = sb_h.tile([C, HW], bf16, tag="hvn%d" % pair)
        nc.vector.tensor_mul(out=hvn, in0=hv, in1=rb)
        hvns[(b, pair)] = hvn

    def end_tail(b):
        pp = ps_acc.tile([C, HW], f32, tag="ha23")
        for pair in range(2):
            pj = pjA_sb if pair == 0 else pjB_sb
            hvn = hvns.pop((b, pair))
            for n in range(2):
                sl = slice(n * NHALF, (n + 1) * NHALF)
                nc.tensor.matmul(pp[:, sl], pj, hvn[:, sl],
                                 start=(pair == 0), stop=(pair == 1))
        out_sb = sb_x.tile([C, HW], f32, tag="out")
        nc.vector.tensor_add(out=out_sb, in0=pp, in1=xb2s.pop(b))
        nc.sync.dma_start(out=y_d[b], in_=out_sb)

    # ---------------- schedule ----------------
    xns = {0: prologue_a(0)}
    prologue_q(0, xns[0])
    prologue_k(0, xns[0])
    prologue_v(0, xns[0])

    for b in range(BPC):
        for pair in range(2):
            for j in range(8):
                unit(b, pair, j)
                key = (pair, j)
                if key == (0, 0) and b > 0:
                    pair_tail(b - 1, 1)
                elif key == (0, 1) and b > 0:
                    end_tail(b - 1)
                elif key == (0, 5) and b + 1 < BPC:
                    xns[b + 1] = prologue_a(b + 1)
                elif key == (0, 7):
                    pair_tail(b, 0)
                elif key == (1, 1) and b + 1 < BPC:
                    prologue_q(b + 1, xns[b + 1])
                elif key == (1, 3) and b + 1 < BPC:
                    prologue_k(b + 1, xns[b + 1])
                elif key == (1, 5) and b + 1 < BPC:
                    prologue_v(b + 1, xns.pop(b + 1))
    pair_tail(BPC - 1, 1)
    end_tail(BPC - 1)


def _get_nc():
    if "nc" not in _CACHE:
        _CACHE["nc"] = _build_nc()
    return _CACHE["nc"]


def _host_prep(inputs):
    import ml_dtypes
    bf = ml_dtypes.bfloat16

    x = np.ascontiguousarray(
        np.asarray(inputs["x"], np.float32).reshape(B, C, HW))
    qkv_w = np.asarray(inputs["qkv_w"], np.float32)
    proj_w = np.asarray(inputs["proj_w"], np.float32)
    proj_b = np.asarray(inputs["proj_b"], np.float32)
    norm_w = np.asarray(inputs["norm_w"], np.float32)
    norm_b = np.asarray(inputs["norm_b"], np.float32)

    w3 = qkv_w.reshape(NH, 3, DH, C)  # rows: h*96 + which*32 + d
    wq = w3[:, 0].reshape(C, C)
    wk = w3[:, 1].reshape(C, C)
    wv = w3[:, 2].reshape(C, C)
    wqT = np.ascontiguousarray(wq.T).astype(bf)  # scale^2 applied in exp
    wkT = np.ascontiguousarray(wk.T).astype(bf)
    wvT = np.ascontiguousarray(wv.T).astype(bf)

    # proj weights permuted to consume the head-pair PSUM row layout:
    # rows 0-31 = first head of pair, rows 64-95 = second head.
    pjA = np.zeros((C, C), np.float32)
    pjB = np.zeros((C, C), np.float32)
    pjA[0:32, :] = proj_w[:, 0:32].T
    pjA[64:96, :] = proj_w[:, 32:64].T
    pjB[0:32, :] = proj_w[:, 64:96].T
    pjB[64:96, :] = proj_w[:, 96:128].T
    pjA = pjA.astype(bf)
    pjB = pjB.astype(bf)

    ebcP = np.zeros((2, C), np.float32)
    ebcP[0, 0:32] = 1.0
    ebcP[1, 64:96] = 1.0
    ebcP = ebcP.astype(bf)

    g1 = np.zeros((C, GROUPS), np.float32)
    g1[np.arange(C), np.arange(C) // 4] = 0.25
    g2 = np.zeros((GROUPS, C), np.float32)
    g2[np.arange(C) // 4, np.arange(C)] = 1.0

    params = dict(
        wqT=wqT, wkT=wkT, wvT=wvT, pjA=pjA, pjB=pjB, ebcP=ebcP,
        pjb=np.ascontiguousarray(proj_b[:, None]),
        nw=np.ascontiguousarray(norm_w[:, None]),
        nb=np.ascontiguousarray(norm_b[:, None]),
        g1=g1, g2=g2,
    )
    in_maps = []
    for i in range(NCORES):
        m = dict(params)
        m["x"] = np.ascontiguousarray(x[i * BPC:(i + 1) * BPC])
        in_maps.append(m)
    return in_maps


def kernel(**inputs):
    global LAST_RESULT
    from concourse.bass_utils import run_bass_kernel_spmd
    in_maps = _host_prep(inputs)
    nc = _get_nc()
    res = run_bass_kernel_spmd(nc, in_maps, list(range(NCORES)), trace=TRACE)
    LAST_RESULT = res
    y = np.concatenate([res.results[i]["y"] for i in range(NCORES)], axis=0)
    return y.reshape(B, C, 32, 32)


# revision 8
# speedup vs baseline: 1.3557x; 1.1098x over previous
"""AttentionBlock Trainium2 Bass kernel (v3: ACT-bound design).

Full inputs -> shard batch over 8 NeuronCores (4 samples each) -> full output.

Design notes (per core, 4 samples):
  The softmax exp is irreducible work for the Activation engine:
  4 samples x 4 heads x 1024x1024 scores / 128 lanes = 131072 ACT columns
  ~= 112-135 us.  Everything else is scheduled to hide underneath it:

  - S = K^T Q per head runs as fp8(e4m3) DoubleRow matmuls (0.5 cyc/col):
    q8/k8 are stored [128, 2, 1024] with the second k-tile zeroed, so the
    natural channel layout is kept and the matmul still gets the 2x rate.
  - exp runs on ACT straight out of PSUM with scale=1/32 (the attention
    scale^2 folded into the activation), output written directly as fp8
    into the kt-slice layout the SV matmul wants.
  - SV (hout + Z via a fp8 ones-column in vt) uses real DoubleRow k-tiles
    (pairs of s-chunks), accumulating 4 heads into two PSUM accumulators
    (two heads share banks at partition offsets 0/64).
  - Z normalization: DVE reciprocal of the Z rows, tiny DMA gather to a
    [2, 1024] tile, PE broadcast, DVE multiply; proj consumes the two
    head-pair tiles directly via two accumulated matmuls with permuted
    weights (no hun reassembly DMA).
  - PSUM: 2-slot S' ring (4 banks) + 2 head-pair accumulators (4 banks);
    the broadcast/proj tiles reuse the accumulator tags.
  - The per-sample prologue (groupnorm, QKV, quantize) is emitted in
    pieces between attention units so every engine queue stays busy.
"""

import numpy as np
from contextlib import ExitStack

B, C, HW = 32, 128, 1024
NH, DH = 4, 32
GROUPS = 32
EPS = 1e-5
NCORES = 8
BPC = B // NCORES  # samples per core
NJP = 4  # s-chunk pairs

_CACHE = {}
TRACE = False
LAST_RESULT = None


def _patch_tile_waits(tile, mybir):
    """This walrus build encodes only one sync-wait slot per instruction;
    Tile can attach several. Split extra waits onto NoOps committed
    immediately before the instruction on the same engine queue
    (in-order => identical semantics)."""
    if getattr(tile.TileContext, "_mm_wait_patched", False):
        return
    orig = tile.TileContext._commit_instruction

    def patched(self, inst, lazy_reg_writes=True):
        si = getattr(inst, "sync_info", None)
        if (not isinstance(inst, mybir.InstNoOp) and si is not None
                and si.on_wait and len(si.on_wait) > 1):
            waits = list(si.on_wait)
            for w in waits[:-1]:
                nop = mybir.InstNoOp(
                    name=self.nc.get_next_instruction_name(),
                    engine=inst.engine,
                    bass_nofuse=True,
                    sync_info=mybir.SyncInfo(on_wait=[w], on_update=[]),
                )
                orig(self, nop, lazy_reg_writes=False)
            inst.sync_info = mybir.SyncInfo(
                on_wait=[waits[-1]], on_update=list(si.on_update))
        return orig(self, inst, lazy_reg_writes)

    tile.TileContext._commit_instruction = patched

    def patched_drain(self, tick_clock, wait_clock):
        # Collect end-of-kernel waits, then hand them out one per SP nop
        # (the drain keeps none); nops precede the teardown barrier on the
        # same queue, so semantics are preserved.
        self.nc.sync.drain()
        sink = self.nc.sync.nop(nofuse=True)
        wait_clock.add_sem_waits(
            sink.ins, tile.ScopedClock({None: tick_clock.global_clock}))
        si = sink.ins.sync_info
        waits = list(si.on_wait) if si and si.on_wait else []
        if len(waits) > 1:
            sink.ins.sync_info = mybir.SyncInfo(
                on_wait=[waits[0]], on_update=list(si.on_update))
            for w in waits[1:]:
                extra = self.nc.sync.nop(nofuse=True)
                extra.ins.sync_info = mybir.SyncInfo(on_wait=[w], on_update=[])

        self.nc.all_engine_barrier()
        assert self.sems is not None
        popped = self.nc._tile_sem_poison_stack.pop()
        assert popped is self._sem_poison
        self.nc.clear_and_free_semaphores(list(self.sems.allocated().values()))
        self.nc.all_engine_barrier()

    tile.TileContext._drain_and_barrier = patched_drain
    tile.TileContext._mm_wait_patched = True


def _build_nc():
    import concourse.bass as bass
    import concourse.tile as tile
    from concourse import mybir

    _patch_tile_waits(tile, mybir)

    f32 = mybir.dt.float32
    bf16 = mybir.dt.bfloat16
    nc = bass.Bass()

    x_d = nc.dram_tensor("x", [BPC, C, HW], f32, kind="ExternalInput")
    wq_d = nc.dram_tensor("wqT", [C, C], bf16, kind="ExternalInput")
    wk_d = nc.dram_tensor("wkT", [C, C], bf16, kind="ExternalInput")
    wv_d = nc.dram_tensor("wvT", [C, C], bf16, kind="ExternalInput")
    pjA_d = nc.dram_tensor("pjA", [C, C], bf16, kind="ExternalInput")
    pjB_d = nc.dram_tensor("pjB", [C, C], bf16, kind="ExternalInput")
    ebc_d = nc.dram_tensor("ebcP", [2, C], bf16, kind="ExternalInput")
    pjb_d = nc.dram_tensor("pjb", [C, 1], f32, kind="ExternalInput")
    nw_d = nc.dram_tensor("nw", [C, 1], f32, kind="ExternalInput")
    nb_d = nc.dram_tensor("nb", [C, 1], f32, kind="ExternalInput")
    g1_d = nc.dram_tensor("g1", [C, GROUPS], f32, kind="ExternalInput")
    g2_d = nc.dram_tensor("g2", [GROUPS, C], f32, kind="ExternalInput")
    y_d = nc.dram_tensor("y", [BPC, C, HW], f32, kind="ExternalOutput")

    with tile.TileContext(nc) as tc:
        with ExitStack() as ctx:
            _body(ctx, tc, mybir, bass,
                  x_d, wq_d, wk_d, wv_d, pjA_d, pjB_d, ebc_d, pjb_d,
                  nw_d, nb_d, g1_d, g2_d, y_d)
    return nc


def _body(ctx, tc, mybir, bass,
          x_d, wq_d, wk_d, wv_d, pjA_d, pjB_d, ebc_d, pjb_d,
          nw_d, nb_d, g1_d, g2_d, y_d):
    nc = tc.nc
    f32 = mybir.dt.float32
    bf16 = mybir.dt.bfloat16
    fp8 = mybir.dt.float8e4
    AF = mybir.ActivationFunctionType
    OP = mybir.AluOpType
    DR = mybir.MatmulPerfMode.DoubleRow
    NHALF = HW // 2

    const = ctx.enter_context(tc.tile_pool(name="const", bufs=1))
    pers = ctx.enter_context(tc.tile_pool(name="pers", bufs=1))
    sb_x = ctx.enter_context(tc.tile_pool(name="sb_x", bufs=2))
    sb_a = ctx.enter_context(tc.tile_pool(name="sb_a", bufs=4))
    sb_qk = ctx.enter_context(tc.tile_pool(name="sb_qk", bufs=2))
    sb_h = ctx.enter_context(tc.tile_pool(name="sb_h", bufs=2))
    sb_sm = ctx.enter_context(tc.tile_pool(name="sb_sm", bufs=2))
    # PSUM: ring 2 x [128,1024]f32 (4 banks) + ha01/ha23 (4 banks) = 8
    ps_ring = ctx.enter_context(tc.tile_pool(name="ps_ring", bufs=2,
                                             space="PSUM"))
    ps_acc = ctx.enter_context(tc.tile_pool(name="ps_acc", bufs=1,
                                            space="PSUM"))

    # ---- constants ----
    wq_sb = const.tile([C, C], bf16, tag="wq")
    wk_sb = const.tile([C, C], bf16, tag="wk")
    wv_sb = const.tile([C, C], bf16, tag="wv")
    pjA_sb = const.tile([C, C], bf16, tag="pjA")
    pjB_sb = const.tile([C, C], bf16, tag="pjB")
    ebc_sb = const.tile([2, C], bf16, tag="ebc")
    pjb_sb = const.tile([C, 1], f32, tag="pjb")
    nw_sb = const.tile([C, 1], f32, tag="nw")
    nb_sb = const.tile([C, 1], f32, tag="nb")
    g1_sb = const.tile([C, GROUPS], f32, tag="g1")
    g2_sb = const.tile([GROUPS, C], f32, tag="g2")
    for dst, src in ((wq_sb, wq_d), (wk_sb, wk_d), (wv_sb, wv_d),
                     (pjA_sb, pjA_d), (pjB_sb, pjB_d), (ebc_sb, ebc_d),
                     (pjb_sb, pjb_d), (nw_sb, nw_d), (nb_sb, nb_d),
                     (g1_sb, g1_d), (g2_sb, g2_d)):
        nc.sync.dma_start(out=dst, in_=src[:])
    eps_sb = const.tile([GROUPS, 1], f32, tag="eps")
    nc.vector.memset(eps_sb, EPS)

    # persistent fp8 operand tiles (double-buffered by sample parity);
    # the kt=1 half of q8/k8 and the ones column of vt8 are write-once
    # vtP[p, pair, j, kt, col]: kt is the head-in-pair index; plane kt=0
    # holds head 2*pair in cols 0-63 (v cols 0-31, Z ones at 32), plane
    # kt=1 holds head 2*pair+1 in cols 64-127 (ones at 96); everything
    # else is zero, so one DoubleRow matmul computes both heads' SV with
    # full 128-partition output and no column tiling.
    vtPs = [pers.tile([C, 2, 8, 2, C], fp8, tag="vtP_%d" % i,
                      name="vtP_%d" % i) for i in range(2)]
    for i in range(2):
        nc.vector.memset(vtPs[i], 0.0)
        nc.vector.memset(vtPs[i][:, :, :, 0, 32:33], 1.0)
        nc.vector.memset(vtPs[i][:, :, :, 1, 96:97], 1.0)

    xb2s = {}

    # ---------------- prologue pieces ----------------
    def prologue_a(b):
        # load + groupnorm stats + affine -> xn
        x_sb = sb_x.tile([C, HW], f32, tag="x")
        nc.sync.dma_start(out=x_sb, in_=x_d[b])
        xb2 = sb_x.tile([C, HW], f32, tag="xb2")
        nc.vector.tensor_scalar(out=xb2, in0=x_sb, scalar1=pjb_sb,
                                scalar2=None, op0=OP.add)
        xb2s[b] = xb2

        st6 = sb_sm.tile([C, 2, 6], f32, tag="st6")
        nc.vector.bn_stats(out=st6[:, 0, :], in_=x_sb[:, 0:512])
        nc.vector.bn_stats(out=st6[:, 1, :], in_=x_sb[:, 512:1024])
        mv = sb_sm.tile([C, 2], f32, tag="mv")
        nc.vector.bn_aggr(out=mv, in_=st6)
        # s2 = [mean_c, mean_c^2 + var_c]
        s2 = sb_sm.tile([C, 2], f32, tag="s2")
        nc.vector.tensor_copy(out=s2[:, 0:1], in_=mv[:, 0:1])
        nc.vector.tensor_mul(out=s2[:, 1:2], in0=mv[:, 0:1], in1=mv[:, 0:1])
        nc.vector.tensor_add(out=s2[:, 1:2], in0=s2[:, 1:2], in1=mv[:, 1:2])
        gp = ps_ring.tile([C, HW], f32, tag="s")
        nc.tensor.matmul(gp[0:GROUPS, 0:2], g1_sb, s2, start=True, stop=True)
        gs = sb_sm.tile([GROUPS, 2], f32, tag="gs")
        nc.vector.tensor_copy(out=gs, in_=gp[0:GROUPS, 0:2])
        # vv = [mu_g, rstd_g]; rstd = exp(-0.5*ln(var+eps))
        vv = sb_sm.tile([GROUPS, 2], f32, tag="vv")
        nc.vector.tensor_mul(out=vv[:, 0:1], in0=gs[:, 0:1], in1=gs[:, 0:1])
        nc.vector.tensor_tensor(out=vv[:, 1:2], in0=gs[:, 1:2], in1=vv[:, 0:1],
                                op=OP.subtract)
        nc.scalar.activation(out=vv[:, 1:2], in_=vv[:, 1:2], func=AF.Ln,
                             bias=eps_sb, scale=1.0)
        nc.scalar.activation(out=vv[:, 1:2], in_=vv[:, 1:2], func=AF.Exp,
                             bias=0.0, scale=-0.5)
        nc.vector.tensor_copy(out=vv[:, 0:1], in_=gs[:, 0:1])
        bc = ps_ring.tile([C, HW], f32, tag="s")
        nc.tensor.matmul(bc[0:C, 0:2], g2_sb, vv, start=True, stop=True)
        aff = sb_sm.tile([C, 2], f32, tag="aff")
        nc.vector.tensor_mul(out=aff[:, 0:1], in0=nw_sb, in1=bc[:, 1:2])
        nc.vector.tensor_mul(out=aff[:, 1:2], in0=bc[:, 0:1], in1=aff[:, 0:1])
        nc.vector.tensor_tensor(out=aff[:, 1:2], in0=nb_sb, in1=aff[:, 1:2],
                                op=OP.subtract)
        xn = sb_x.tile([C, HW], bf16, tag="xn")
        nc.vector.tensor_scalar(out=xn, in0=x_sb,
                                scalar1=aff[:, 0:1], scalar2=aff[:, 1:2],
                                op0=OP.mult, op1=OP.add)
        return xn

    qks = {}

    def prologue_q(b, xn):
        qp = ps_ring.tile([C, HW], f32, tag="s")
        for n in range(2):
            sl = slice(n * NHALF, (n + 1) * NHALF)
            nc.tensor.matmul(qp[:, sl], wq_sb, xn[:, sl], start=True, stop=True)
        q_sb = sb_qk.tile([C, HW], bf16, tag="q")
        nc.vector.tensor_copy(out=q_sb, in_=qp)
        qks[(b, "q")] = q_sb

    def prologue_k(b, xn):
        kp = ps_ring.tile([C, HW], f32, tag="s")
        for n in range(2):
            sl = slice(n * NHALF, (n + 1) * NHALF)
            nc.tensor.matmul(kp[:, sl], wk_sb, xn[:, sl], start=True, stop=True)
        k_sb = sb_qk.tile([C, HW], bf16, tag="k")
        nc.vector.tensor_copy(out=k_sb, in_=kp)
        qks[(b, "k")] = k_sb

    def prologue_v(b, xn):
        vp = ps_ring.tile([C, HW], f32, tag="s")
        vtP = vtPs[b % 2]
        for j in range(8):
            nc.tensor.matmul(vp[:, j * 128:(j + 1) * 128],
                             xn[:, j * 128:(j + 1) * 128], wv_sb,
                             start=True, stop=True)
        for j in range(8):
            src_v = vp[:, j * 128:(j + 1) * 128].rearrange(
                "p (pr hi d) -> p pr hi d", hi=2, d=DH)
            nc.vector.tensor_copy(out=vtP[:, :, j, 0, 0:DH],
                                  in_=src_v[:, :, 0, :])
            nc.vector.tensor_copy(out=vtP[:, :, j, 1, 64:64 + DH],
                                  in_=src_v[:, :, 1, :])

    # ---------------- attention unit ----------------
    ha_cur = {}

    pending_sv = []

    def flush_sv():
        # SV matmuls are issued one unit late so they never block the next
        # unit's S' matmuls in the PE's in-order queue (their aP/exp deps
        # are already met by the time they issue)
        while pending_sv:
            b, pair, j, aP = pending_sv.pop(0)
            vtP = vtPs[b % 2]
            if j == 0:
                ha_cur[pair] = ps_acc.tile(
                    [C, HW], f32, tag="ha01" if pair == 0 else "ha23",
                    name="ha%d" % pair)
            ha = ha_cur[pair]
            for n in range(2):
                sl = slice(n * NHALF, (n + 1) * NHALF)
                nc.tensor.matmul(
                    ha[:, sl],
                    vtP[:, pair, j, :, :],
                    aP[:, :, sl],
                    start=(j == 0), stop=(j == 7),
                    tile_position=(0, 0),
                    perf_mode=DR)

    def unit(b, pair, j):
        q_sb, k_sb = qks[(b, "q")], qks[(b, "k")]
        aP = sb_a.tile([C, 2, HW], fp8, tag="a", name="aP")
        sps = [ps_ring.tile([C, HW], f32, tag="s", name="sp%d" % hi)
               for hi in range(2)]
        # issue the two heads' S' matmuls interleaved: they sit in
        # different 32-row PE bands (tile_position) and stream
        # concurrently in the array
        for n in range(2):
            sl = slice(n * NHALF, (n + 1) * NHALF)
            for hi in range(2):
                h = 2 * pair + hi
                hp = slice(32 * h, 32 * h + 32)
                nc.tensor.matmul(
                    sps[hi][:, sl],
                    k_sb[hp, 128 * j:128 * (j + 1)],
                    q_sb[hp, sl],
                    start=True, stop=True,
                    tile_position=(32 * h, 0))
        flush_sv()
        for hi in range(2):
            nc.scalar.activation(out=aP[:, hi, :], in_=sps[hi], func=AF.Exp,
                                 bias=0.0, scale=1.0 / 32.0)
        pending_sv.append((b, pair, j, aP))

    # ---------------- tails ----------------
    hvns = {}

    def pair_tail(b, pair):
        ha = ha_cur[pair]
        hv = sb_h.tile([C, HW], bf16, tag="hv%d" % pair)
        nc.vector.tensor_copy(out=hv, in_=ha)
        # 1/Z: spread the two Z rows across partitions so the DVE
        # reciprocal runs at free-size 16 instead of 1024
        zp = sb_h.tile([C, 2, 8], bf16, tag="zp%d" % pair)
        nc.sync.dma_start(out=zp[:, 0, :], in_=hv[32:33, :])
        nc.sync.dma_start(out=zp[:, 1, :], in_=hv[96:97, :])
        rp = sb_h.tile([C, 2, 8], bf16, tag="rp%d" % pair)
        with nc.allow_low_precision(reason="1/Z in bf16; gate 2e-2"):
            nc.vector.reciprocal(out=rp, in_=zp)
        ral = sb_h.tile([2, HW], bf16, tag="ral%d" % pair)
        nc.sync.dma_start(out=ral[0:1, :], in_=rp[:, 0, :])
        nc.sync.dma_start(out=ral[1:2, :], in_=rp[:, 1, :])
        # broadcast 1/Z to the hv row layout, reusing the accumulator banks
        rb = ps_acc.tile([C, HW], f32, tag="ha01" if pair == 0 else "ha23")
        for n in range(2):
            sl = slice(n * NHALF, (n + 1) * NHALF)
            nc.tensor.matmul(rb[:, sl], ebc_sb, ral[:, sl],
                             start=True, stop=True)
        hvn = sb_h.tile([C, HW], bf16, tag="hvn%d" % pair)
        nc.vector.tensor_mul(out=hvn, in0=hv, in1=rb)
        hvns[(b, pair)] = hvn

    def end_tail(b):
        pp = ps_acc.tile([C, HW], f32, tag="ha23")
        for pair in range(2):
            pj = pjA_sb if pair == 0 else pjB_sb
            hvn = hvns.pop((b, pair))
            for n in range(2):
                sl = slice(n * NHALF, (n + 1) * NHALF)
                nc.tensor.matmul(pp[:, sl], pj, hvn[:, sl],
                                 start=(pair == 0), stop=(pair == 1))
        out_sb = sb_x.tile([C, HW], f32, tag="out")
        nc.vector.tensor_add(out=out_sb, in0=pp, in1=xb2s.pop(b))
        nc.sync.dma_start(out=y_d[b], in_=out_sb)

    # ---------------- schedule ----------------
    xns = {0: prologue_a(0)}
    prologue_q(0, xns[0])
    prologue_k(0, xns[0])
    prologue_v(0, xns[0])

    for b in range(BPC):
        for pair in range(2):
            for j in range(8):
                unit(b, pair, j)
                key = (pair, j)
                if key == (0, 0) and b > 0:
                    pair_tail(b - 1, 1)
                elif key == (0, 1) and b > 0:
                    end_tail(b - 1)
                elif key == (0, 5) and b + 1 < BPC:
                    xns[b + 1] = prologue_a(b + 1)
                elif key == (1, 0):
                    pair_tail(b, 0)
                elif key == (1, 1) and b + 1 < BPC:
                    prologue_q(b + 1, xns[b + 1])
                elif key == (1, 3) and b + 1 < BPC:
                    prologue_k(b + 1, xns[b + 1])
                elif key == (1, 5) and b + 1 < BPC:
                    prologue_v(b + 1, xns.pop(b + 1))
    flush_sv()
    pair_tail(BPC - 1, 1)
    end_tail(BPC - 1)


def _get_nc():
    if "nc" not in _CACHE:
        _CACHE["nc"] = _build_nc()
    return _CACHE["nc"]


def _host_prep(inputs):
    import ml_dtypes
    bf = ml_dtypes.bfloat16

    x = np.ascontiguousarray(
        np.asarray(inputs["x"], np.float32).reshape(B, C, HW))
    qkv_w = np.asarray(inputs["qkv_w"], np.float32)
    proj_w = np.asarray(inputs["proj_w"], np.float32)
    proj_b = np.asarray(inputs["proj_b"], np.float32)
    norm_w = np.asarray(inputs["norm_w"], np.float32)
    norm_b = np.asarray(inputs["norm_b"], np.float32)

    w3 = qkv_w.reshape(NH, 3, DH, C)  # rows: h*96 + which*32 + d
    wq = w3[:, 0].reshape(C, C)
    wk = w3[:, 1].reshape(C, C)
    wv = w3[:, 2].reshape(C, C)
    wqT = np.ascontiguousarray(wq.T).astype(bf)  # scale^2 applied in exp
    wkT = np.ascontiguousarray(wk.T).astype(bf)
    wvT = np.ascontiguousarray(wv.T).astype(bf)

    # proj weights permuted to consume the head-pair PSUM row layout:
    # rows 0-31 = first head of pair, rows 64-95 = second head.
    pjA = np.zeros((C, C), np.float32)
    pjB = np.zeros((C, C), np.float32)
    pjA[0:32, :] = proj_w[:, 0:32].T
    pjA[64:96, :] = proj_w[:, 32:64].T
    pjB[0:32, :] = proj_w[:, 64:96].T
    pjB[64:96, :] = proj_w[:, 96:128].T
    pjA = pjA.astype(bf)
    pjB = pjB.astype(bf)

    ebcP = np.zeros((2, C), np.float32)
    ebcP[0, 0:32] = 1.0
    ebcP[1, 64:96] = 1.0
    ebcP = ebcP.astype(bf)

    g1 = np.zeros((C, GROUPS), np.float32)
    g1[np.arange(C), np.arange(C) // 4] = 0.25
    g2 = np.zeros((GROUPS, C), np.float32)
    g2[np.arange(C) // 4, np.arange(C)] = 1.0

    params = dict(
        wqT=wqT, wkT=wkT, wvT=wvT, pjA=pjA, pjB=pjB, ebcP=ebcP,
        pjb=np.ascontiguousarray(proj_b[:, None]),
        nw=np.ascontiguousarray(norm_w[:, None]),
        nb=np.ascontiguousarray(norm_b[:, None]),
        g1=g1, g2=g2,
    )
    in_maps = []
    for i in range(NCORES):
        m = dict(params)
        m["x"] = np.ascontiguousarray(x[i * BPC:(i + 1) * BPC])
        in_maps.append(m)
    return in_maps


def kernel(**inputs):
    global LAST_RESULT
    from concourse.bass_utils import run_bass_kernel_spmd
    in_maps = _host_prep(inputs)
    nc = _get_nc()
    res = run_bass_kernel_spmd(nc, in_maps, list(range(NCORES)), trace=TRACE)
    LAST_RESULT = res
    y = np.concatenate([res.results[i]["y"] for i in range(NCORES)], axis=0)
    return y.reshape(B, C, 32, 32)
